# revision 22
# baseline (speedup 1.0000x reference)
"""fp8-e4m3 DoubleRow variant of the AlignmentContrastiveLoss kernel.

Same structure as kernel.py, but the einsum runs in fp8e4 with
perf_mode=DoubleRow (virtual 128x256 PE array): 4 accumulating K=256
matmuls per (w, half) instead of 8 K=128 bf16 ones.  PSUM stays fp32, so
the reduction path (and its precision) is unchanged; only the operand
quantization differs (measured fro rel err ~2.8e-3 vs 1.7e-4 for bf16,
against a 2e-2 gate).
"""

import numpy as np
import ml_dtypes

import concourse.bacc as bacc
import concourse.mybir as mybir
import concourse.tile as tile
from concourse.bass_utils import run_bass_kernel_spmd

B = 128
L_IM, L_S, D = 50, 40, 1024
R = L_IM - 1     # 49
W = L_S - 3      # 37
NCORES = 8
IPC = B // NCORES            # 16
N = IPC * R                  # 784
NH = N // 2                  # 392
IH = IPC // 2                # 8
K4 = D // 256                # 4 double-row contraction chunks
FP8 = mybir.dt.float8e4
BF16 = mybir.dt.bfloat16
F32 = mybir.dt.float32
X = mybir.AxisListType.X
DR = mybir.MatmulPerfMode.DoubleRow

_NC_CACHE = None


def _build():
    nc = bacc.Bacc("TRN2", target_bir_lowering=False, debug=False,
                   num_devices=NCORES)
    # sT layout: [dk, (w, k4, h, j)] with d = k4*256 + h*128 + dk
    sT = nc.dram_tensor("sT", [128, W * K4 * 2 * B], FP8, kind="ExternalInput")
    # imT layout: [dk, (k4, h, ir)]
    imT = nc.dram_tensor("imT", [128, K4 * 2 * N], FP8, kind="ExternalInput")
    out = nc.dram_tensor("out", [B, IPC], F32, kind="ExternalOutput")

    with tile.TileContext(nc) as tc:
        with (
            tc.tile_pool(name="persist", bufs=1) as persist,
            tc.tile_pool(name="sw", bufs=6) as swpool,
            tc.tile_pool(name="ps", bufs=3, space="PSUM") as pspool,
            tc.tile_pool(name="warm", bufs=1, space="PSUM") as warmpool,
            tc.tile_pool(name="al", bufs=3) as alpool,
        ):
            # --- startup DMAs -------------------------------------------
            s_tiles = [None] * W
            s_tiles[0] = swpool.tile([128, K4 * 2 * 128], FP8, tag="s_w",
                                     name="s_w0")
            nc.sync.dma_start(s_tiles[0][:], sT.ap()[:, 0:K4 * 2 * B])

            # im chunks: one [128, 2, N] fp8 tile per K=256 chunk
            imt = [persist.tile([128, 2 * N], FP8, tag=f"imt{c}",
                                name=f"imt{c}") for c in range(K4)]
            nc.sync.dma_start(imt[0][:], imT.ap()[:, 0:2 * N])
            nc.scalar.dma_start(imt[1][:], imT.ap()[:, 2 * N:4 * N])
            nc.sync.dma_start(imt[2][:], imT.ap()[:, 4 * N:6 * N])
            nc.scalar.dma_start(imt[3][:], imT.ap()[:, 6 * N:8 * N])

            def im_c(c, half):      # [128, 2, 392] moving operand slice
                return imt[c][:].rearrange("p (h n) -> p h n", h=2)[
                    :, :, half * NH:(half + 1) * NH]

            def s_c(s_w, c):        # [128, 2, 128] stationary slice
                return s_w[:].rearrange("p (c h j) -> p c h j", c=K4, h=2)[
                    :, c, :, :]

            # --- PE warm-up ---------------------------------------------
            dummy = persist.tile([128, 128], mybir.dt.bfloat16)
            nc.vector.memset(dummy[:], 0)
            warm = warmpool.tile([128, 128], F32)
            for _ in range(38):
                nc.tensor.matmul(warm[:], dummy[:], dummy[:],
                                 start=True, stop=True)

            runmax = persist.tile([128, N], BF16)
            maxr = persist.tile([128, IPC, W], F32)
            term1a = persist.tile([128, IPC], F32)

            for w in range(W):
                if w > 0:
                    s_tiles[w] = swpool.tile([128, K4 * 2 * 128], FP8,
                                             tag="s_w", name=f"s_w{w}")
                    nc.sync.dma_start(
                        s_tiles[w][:],
                        sT.ap()[:, w * K4 * 2 * B:(w + 1) * K4 * 2 * B])
                s_w = s_tiles[w]

                last = w == W - 1
                if w >= W - 3:
                    psh = [pspool.tile([128, 2, 512], F32, tag="ps",
                                       name=f"psl{w}_{h}") for h in (0, 1)]
                    term1 = persist.tile([128, IPC], F32)
                    term2 = persist.tile([128, IPC], F32)
                    for half in (0, 1):
                        for c in range(K4):
                            nc.tensor.matmul(psh[half][:, 0, 0:NH],
                                             s_c(s_w, c), im_c(c, half),
                                             start=(c == 0), stop=(c == K4 - 1),
                                             perf_mode=DR)
                        lo, hi = half * IH, (half + 1) * IH
                        sl = slice(half * NH, (half + 1) * NH)
                        alh = alpool.tile([128, NH], BF16, tag="al",
                                          name=f"alh{w}_{half}")
                        nc.scalar.copy(alh[:], psh[half][:, 0, 0:NH])
                        nc.vector.reduce_max(
                            maxr[:, lo:hi, w],
                            alh[:].rearrange("p (i r) -> p i r", r=R),
                            axis=X)
                        nc.vector.tensor_max(runmax[:, sl], runmax[:, sl],
                                             alh[:])
                        if last:
                            nc.vector.reduce_sum(term1[:, lo:hi],
                                                 maxr[:, lo:hi, 32:W], axis=X)
                            nc.vector.reduce_sum(
                                term2[:, lo:hi],
                                runmax[:, sl].rearrange("p (i r) -> p i r",
                                                        r=R),
                                axis=X)
                    if last:
                        res = persist.tile([128, IPC], F32)
                        nc.vector.tensor_add(res[:], term1[:], term2[:])
                        nc.vector.tensor_add(res[:], res[:], term1a[:])
                        nc.sync.dma_start(out.ap()[:], res[:])
                else:
                    ps = pspool.tile([128, 2, 512], F32)
                    for c in range(K4):
                        lhsT = s_c(s_w, c)
                        for half in (0, 1):
                            nc.tensor.matmul(ps[:, half, 0:NH],
                                             lhsT, im_c(c, half),
                                             start=(c == 0), stop=(c == K4 - 1),
                                             perf_mode=DR)
                    # ScalarE evacuates PSUM as bf16 so both DVE ops run in
                    # SBUF/bf16 mode (TT at 2x) and PSUM frees early.
                    if w == 0:
                        # ScalarE seeds runmax directly; reduce reads PSUM.
                        nc.scalar.copy(
                            runmax[:].rearrange("p (h n) -> p h n", h=2),
                            ps[:, :, 0:NH])
                        nc.vector.reduce_max(
                            maxr[:, :, w],
                            ps[:, :, 0:NH].rearrange("p h (i r) -> p h i r",
                                                     r=R),
                            axis=X)
                    else:
                        al = alpool.tile([128, N], BF16, tag="al",
                                         name=f"al{w}")
                        nc.scalar.copy(
                            al[:].rearrange("p (h n) -> p h n", h=2),
                            ps[:, :, 0:NH])
                        nc.vector.tensor_max(runmax[:], runmax[:], al[:])
                        nc.vector.reduce_max(
                            maxr[:, :, w],
                            al[:].rearrange("p (i r) -> p i r", r=R),
                            axis=X)
                    if w == 31:
                        nc.vector.reduce_sum(term1a[:], maxr[:, :, 0:32],
                                             axis=X)

    nc.compile()
    return nc


def _get_nc():
    global _NC_CACHE
    if _NC_CACHE is None:
        _NC_CACHE = _build()
    return _NC_CACHE


def kernel(im_set, s_seq, im_len, s_len):
    im_set = np.asarray(im_set, dtype=np.float32)
    s_seq = np.asarray(s_seq, dtype=np.float32)
    im_len = np.asarray(im_len).astype(np.int64)
    s_len = np.asarray(s_len).astype(np.int64)

    im = im_set[:, 1:, :].copy()
    s = s_seq[:, 1:-2, :].copy()
    il = im_len - 1
    sl = s_len - 3
    im *= (np.arange(R)[None, :] < il[:, None])[:, :, None]
    s *= (np.arange(W)[None, :] < sl[:, None])[:, :, None]

    # sT[dk, w, k4, h, j] = s[j, w, k4*256 + h*128 + dk]
    sT = (s.transpose(2, 1, 0)                  # [D, W, B]
          .reshape(K4, 2, 128, W, B)            # [k4, h, dk, w, j]
          .transpose(2, 3, 0, 1, 4)             # [dk, w, k4, h, j]
          .reshape(128, W * K4 * 2 * B)
          .astype(ml_dtypes.float8_e4m3))

    in_maps = []
    for c in range(NCORES):
        im_cc = im[c * IPC:(c + 1) * IPC]
        imT = (im_cc.reshape(N, D)
               .T                               # [D, N]
               .reshape(K4, 2, 128, N)          # [k4, h, dk, ir]
               .transpose(2, 0, 1, 3)           # [dk, k4, h, ir]
               .reshape(128, K4 * 2 * N)
               .astype(ml_dtypes.float8_e4m3))
        in_maps.append({"sT": sT, "imT": np.ascontiguousarray(imT)})

    nc = _get_nc()
    # The accelerator sporadically reports NRT_EXEC_UNIT_UNRECOVERABLE on the
    # first execution of a freshly loaded NEFF; it recovers after a pause.
    import time
    res = None
    for attempt in range(4):
        try:
            res = run_bass_kernel_spmd(nc, in_maps,
                                       core_ids=list(range(NCORES)))
            break
        except Exception:
            if attempt == 3:
                raise
            time.sleep(15 * (attempt + 1))

    full = np.empty((B, B), dtype=np.float32)
    for c in range(NCORES):
        full[c * IPC:(c + 1) * IPC, :] = res.results[c]["out"].T
    return full


# revision 24
# speedup vs baseline: 1.1678x; 1.1678x over previous
"""fp8-e4m3 DoubleRow variant of the AlignmentContrastiveLoss kernel.

Same structure as kernel.py, but the einsum runs in fp8e4 with
perf_mode=DoubleRow (virtual 128x256 PE array): 4 accumulating K=256
matmuls per (w, half) instead of 8 K=128 bf16 ones.  PSUM stays fp32, so
the reduction path (and its precision) is unchanged; only the operand
quantization differs (measured fro rel err ~2.8e-3 vs 1.7e-4 for bf16,
against a 2e-2 gate).
"""

import numpy as np
import ml_dtypes

import concourse.bacc as bacc
import concourse.mybir as mybir
import concourse.tile as tile
from concourse.bass_utils import run_bass_kernel_spmd

B = 128
L_IM, L_S, D = 50, 40, 1024
R = L_IM - 1     # 49
W = L_S - 3      # 37
NCORES = 8
IPC = B // NCORES            # 16
N = IPC * R                  # 784
NH = N // 2                  # 392
IH = IPC // 2                # 8
K4 = D // 256                # 4 double-row contraction chunks
FP8 = mybir.dt.float8e4
BF16 = mybir.dt.bfloat16
F32 = mybir.dt.float32
X = mybir.AxisListType.X
DR = mybir.MatmulPerfMode.DoubleRow

_NC_CACHE = None


def _build():
    nc = bacc.Bacc("TRN2", target_bir_lowering=False, debug=False,
                   num_devices=NCORES)
    # sT layout: [dk, (w, k4, h, j)] with d = k4*256 + h*128 + dk
    sT = nc.dram_tensor("sT", [128, W * K4 * 2 * B], FP8, kind="ExternalInput")
    # imT layout: [dk, (k4, h, ir)]
    imT = nc.dram_tensor("imT", [128, K4 * 2 * N], FP8, kind="ExternalInput")
    out = nc.dram_tensor("out", [B, IPC], F32, kind="ExternalOutput")

    with tile.TileContext(nc) as tc:
        with (
            tc.tile_pool(name="persist", bufs=1) as persist,
            tc.tile_pool(name="sw", bufs=6) as swpool,
            tc.tile_pool(name="ps", bufs=3, space="PSUM") as pspool,
            tc.tile_pool(name="warm", bufs=1, space="PSUM") as warmpool,
            tc.tile_pool(name="al", bufs=3) as alpool,
        ):
            # --- startup DMAs -------------------------------------------
            s_tiles = [None] * W
            s_tiles[0] = swpool.tile([128, K4 * 2 * 128], FP8, tag="s_w",
                                     name="s_w0")
            nc.sync.dma_start(s_tiles[0][:], sT.ap()[:, 0:K4 * 2 * B])

            # im chunks: one [128, 2, N] fp8 tile per K=256 chunk
            imt = [persist.tile([128, 2 * N], FP8, tag=f"imt{c}",
                                name=f"imt{c}") for c in range(K4)]
            nc.sync.dma_start(imt[0][:], imT.ap()[:, 0:2 * N])
            nc.scalar.dma_start(imt[1][:], imT.ap()[:, 2 * N:4 * N])
            nc.sync.dma_start(imt[2][:], imT.ap()[:, 4 * N:6 * N])
            nc.scalar.dma_start(imt[3][:], imT.ap()[:, 6 * N:8 * N])

            def im_c(c, half):      # [128, 2, 392] moving operand slice
                return imt[c][:].rearrange("p (h n) -> p h n", h=2)[
                    :, :, half * NH:(half + 1) * NH]

            def s_c(s_w, c):        # [128, 2, 128] stationary slice
                return s_w[:].rearrange("p (c h j) -> p c h j", c=K4, h=2)[
                    :, c, :, :]

            # --- PE warm-up ---------------------------------------------
            dummy = persist.tile([128, 128], mybir.dt.bfloat16)
            nc.vector.memset(dummy[:], 0)
            warm = warmpool.tile([128, 128], F32)
            for _ in range(38):
                nc.tensor.matmul(warm[:], dummy[:], dummy[:],
                                 start=True, stop=True)

            runmax = persist.tile([128, N], BF16)
            maxr = persist.tile([128, IPC, W], F32)
            term1a = persist.tile([128, IPC], F32)

            for w in range(W):
                if w > 0:
                    s_tiles[w] = swpool.tile([128, K4 * 2 * 128], FP8,
                                             tag="s_w", name=f"s_w{w}")
                    nc.sync.dma_start(
                        s_tiles[w][:],
                        sT.ap()[:, w * K4 * 2 * B:(w + 1) * K4 * 2 * B])
                s_w = s_tiles[w]

                last = w == W - 1
                if w >= W - 3:
                    psh = [pspool.tile([128, 2, 512], F32, tag="ps",
                                       name=f"psl{w}_{h}") for h in (0, 1)]
                    term1 = persist.tile([128, IPC], F32)
                    term2 = persist.tile([128, IPC], F32)
                    for half in (0, 1):
                        for c in range(K4):
                            nc.tensor.matmul(psh[half][:, 0, 0:NH],
                                             s_c(s_w, c), im_c(c, half),
                                             start=(c == 0), stop=(c == K4 - 1),
                                             perf_mode=DR)
                        lo, hi = half * IH, (half + 1) * IH
                        sl = slice(half * NH, (half + 1) * NH)
                        alh = alpool.tile([128, NH], BF16, tag="al",
                                          name=f"alh{w}_{half}")
                        nc.scalar.copy(alh[:], psh[half][:, 0, 0:NH])
                        nc.vector.reduce_max(
                            maxr[:, lo:hi, w],
                            alh[:].rearrange("p (i r) -> p i r", r=R),
                            axis=X)
                        nc.vector.tensor_max(runmax[:, sl], runmax[:, sl],
                                             alh[:])
                        if last:
                            nc.vector.reduce_sum(term1[:, lo:hi],
                                                 maxr[:, lo:hi, 32:W], axis=X)
                            nc.vector.reduce_sum(
                                term2[:, lo:hi],
                                runmax[:, sl].rearrange("p (i r) -> p i r",
                                                        r=R),
                                axis=X)
                    if last:
                        res = persist.tile([128, IPC], F32)
                        nc.vector.tensor_add(res[:], term1[:], term2[:])
                        nc.vector.tensor_add(res[:], res[:], term1a[:])
                        nc.sync.dma_start(out.ap()[:], res[:])
                else:
                    ps = pspool.tile([128, 2, 512], F32)
                    for c in range(K4):
                        lhsT = s_c(s_w, c)
                        for half in (0, 1):
                            nc.tensor.matmul(ps[:, half, 0:NH],
                                             lhsT, im_c(c, half),
                                             start=(c == 0), stop=(c == K4 - 1),
                                             perf_mode=DR)
                    # ScalarE evacuates PSUM as bf16 so both DVE ops run in
                    # SBUF/bf16 mode (TT at 2x) and PSUM frees early.
                    if w == 0:
                        # ScalarE seeds runmax directly; reduce reads PSUM.
                        nc.scalar.copy(
                            runmax[:].rearrange("p (h n) -> p h n", h=2),
                            ps[:, :, 0:NH])
                        nc.vector.reduce_max(
                            maxr[:, :, w],
                            ps[:, :, 0:NH].rearrange("p h (i r) -> p h i r",
                                                     r=R),
                            axis=X)
                    else:
                        al = alpool.tile([128, N], BF16, tag="al",
                                         name=f"al{w}")
                        nc.scalar.copy(
                            al[:].rearrange("p (h n) -> p h n", h=2),
                            ps[:, :, 0:NH])
                        nc.vector.tensor_max(runmax[:], runmax[:], al[:])
                        nc.vector.reduce_max(
                            maxr[:, :, w],
                            al[:].rearrange("p (i r) -> p i r", r=R),
                            axis=X)
                    if w == 31:
                        nc.vector.reduce_sum(term1a[:], maxr[:, :, 0:32],
                                             axis=X)

    nc.compile()
    return nc


def _get_nc():
    global _NC_CACHE
    if _NC_CACHE is None:
        _NC_CACHE = _build()
    return _NC_CACHE


def kernel(im_set, s_seq, im_len, s_len):
    im_set = np.asarray(im_set, dtype=np.float32)
    s_seq = np.asarray(s_seq, dtype=np.float32)
    im_len = np.asarray(im_len).astype(np.int64)
    s_len = np.asarray(s_len).astype(np.int64)

    im = im_set[:, 1:, :].copy()
    s = s_seq[:, 1:-2, :].copy()
    il = im_len - 1
    sl = s_len - 3
    im *= (np.arange(R)[None, :] < il[:, None])[:, :, None]
    s *= (np.arange(W)[None, :] < sl[:, None])[:, :, None]

    # sT[dk, w, k4, h, j] = s[j, w, k4*256 + h*128 + dk]
    sT = (s.transpose(2, 1, 0)                  # [D, W, B]
          .reshape(K4, 2, 128, W, B)            # [k4, h, dk, w, j]
          .transpose(2, 3, 0, 1, 4)             # [dk, w, k4, h, j]
          .reshape(128, W * K4 * 2 * B)
          .astype(ml_dtypes.float8_e4m3))

    in_maps = []
    for c in range(NCORES):
        im_cc = im[c * IPC:(c + 1) * IPC]
        imT = (im_cc.reshape(N, D)
               .T                               # [D, N]
               .reshape(K4, 2, 128, N)          # [k4, h, dk, ir]
               .transpose(2, 0, 1, 3)           # [dk, k4, h, ir]
               .reshape(128, K4 * 2 * N)
               .astype(ml_dtypes.float8_e4m3))
        in_maps.append({"sT": sT, "imT": np.ascontiguousarray(imT)})

    nc = _get_nc()
    # The accelerator sporadically reports NRT_EXEC_UNIT_UNRECOVERABLE on the
    # first execution of a freshly loaded NEFF; it recovers after a pause.
    import time
    res = None
    for attempt in range(4):
        try:
            res = run_bass_kernel_spmd(nc, in_maps,
                                       core_ids=list(range(NCORES)))
            break
        except Exception:
            if attempt == 3:
                raise
            time.sleep(15 * (attempt + 1))

    full = np.empty((B, B), dtype=np.float32)
    for c in range(NCORES):
        full[c * IPC:(c + 1) * IPC, :] = res.results[c]["out"].T
    return full
